# revision 34
# baseline (speedup 1.0000x reference)
"""DotProductAttention kernel for 8x TRN2 NeuronCores (SPMD, data-parallel over batch).

Math (per batch b):
    qp = q @ Wq.T ; kp = k @ Wk.T ; vp = v @ Wv.T          # [S, P]
    s  = qp @ kp.T / sqrt(P)  (+ mask bias over k)          # [S, S]
    e  = exp(s)                                             # no max-sub: s ~ N(0,1)
    out = (e @ vp) / sum_k(e)  @ Wout.T                     # [S, D]

Device layout trick: everything is computed transposed-by-contraction so no
on-chip transposes are needed:
    qpT/kpT [P, S]  <- lhsT=W^T[d,p], rhs=q^T[d,s]
    sT      [k, q]  <- lhsT=kpT,      rhs=qpT      (softmax bias = per-partition!)
    aT      [p, q]  <- lhsT=vp[k,p],  rhs=expT[k,q]
    out     [q, d]  <- lhsT=aT[p,q],  rhs=Wout^T[p,d]
The softmax denominator is computed as DVE partial sums over k_out (fused into
the scores phase) plus one tiny PE matmul per 128-q chunk for the k_in partition
sum; the 1/denom scale is fused into the PSUM->SBUF copy of the output.

Host pre-processing: transpose q/k/v to [B, D, S], cast to fp16 (PSUM accumulation
stays fp32; measured absmax rel-err of the fp16 pipeline vs fp32 reference ~4e-4).
"""

import sys

sys.path.insert(0, "/opt/trn_rl_repo")

import numpy as np

import concourse.bass as bass  # noqa: F401  (registers engine classes)
import concourse.mybir as mybir
import concourse.tile as tile
from concourse import bacc
from concourse.bass_utils import run_bass_kernel_spmd

N_CORES = 8
B, Q, K, D, P = 16, 2048, 2048, 1024, 512
B_LOC = B // N_CORES  # batches per core
SCALE = float(1.0 / np.sqrt(np.float32(P)))
MASK_NEG = -1.0e9

F16 = mybir.dt.float16
F32 = mybir.dt.float32

RAW_BUFS = 2  # one [128, 8, 1024] column-group live + one prefetching

# Filled by kernel() when profiling is enabled (PROFILE env var / profile=True).
last_exec_time_ns = None

_cached = {}


def _emit(nc, tc):
    qT_t = nc.dram_tensor("qT", [B_LOC, D, Q], F16, kind="ExternalInput")
    kT_t = nc.dram_tensor("kT", [B_LOC, D, K], F16, kind="ExternalInput")
    vT_t = nc.dram_tensor("vT", [B_LOC, D, K], F16, kind="ExternalInput")
    # weights are host-preblocked into the exact SBUF layouts
    wq_t = nc.dram_tensor("wq", [128, 8, P], F16, kind="ExternalInput")
    wk_t = nc.dram_tensor("wk", [128, 8, P], F16, kind="ExternalInput")
    wv_t = nc.dram_tensor("wv", [128, 8, P], F16, kind="ExternalInput")
    wo_t = nc.dram_tensor("wo", [128, 4, D], F16, kind="ExternalInput")
    mb_t = nc.dram_tensor("mb", [128, B_LOC * 16], F32, kind="ExternalInput")
    out_t = nc.dram_tensor("out", [B_LOC, Q, D], F32, kind="ExternalOutput")
    scr_t = nc.dram_tensor("scr", [1, 1], F32, kind="ExternalOutput")

    Exp = mybir.ActivationFunctionType.Exp
    PSUM = bass.MemorySpace.PSUM

    with (
        tc.tile_pool(name="wpool", bufs=1) as wpool,
        tc.tile_pool(name="cpool", bufs=1) as cpool,
        tc.tile_pool(name="proj", bufs=1) as projpool,
        tc.tile_pool(name="raw", bufs=RAW_BUFS) as rawpool,
        tc.tile_pool(name="expp", bufs=1) as exppool,
        tc.tile_pool(name="part", bufs=2) as partpool,
        tc.tile_pool(name="outp", bufs=3) as outpool,
        tc.tile_pool(name="ps", bufs=4, space=PSUM) as ps,
        tc.tile_pool(name="pa", bufs=3, space=PSUM) as pa,
        tc.tile_pool(name="pd", bufs=1, space=PSUM) as pd,
    ):
        # ---- constants / weights (persist across batches)
        wq_sb = wpool.tile([128, 8, P], F16, tag="wq")
        wk_sb = wpool.tile([128, 8, P], F16, tag="wk")
        wv_sb = wpool.tile([128, 8, P], F16, tag="wv")
        wo_sb = wpool.tile([128, 4, D], F16, tag="wo")
        mb_sb = cpool.tile([128, B_LOC * 16], F32, tag="mb")
        ones_sb = cpool.tile([128, 1], F32, tag="ones")
        nc.vector.memset(ones_sb[:], 1.0)
        # weight DMAs are emitted lazily (just before first use) so the sync
        # queue lands stage-A data for the first matmuls as early as possible

        # Warm the PE clock (HAM un-throttle needs ~3.4us of sustained matmul
        # activity) with dummy matmuls while the head DMAs stream in; without
        # this the first ~70 real matmuls run at 1.2GHz instead of 2.4GHz.
        scratch = cpool.tile([128, 512], F16, tag="scratch")
        nc.vector.memset(scratch[:], 0.0)
        warm_ps = ps.tile([128, 512], F32, tag="ps")
        for _ in range(12):
            nc.tensor.matmul(warm_ps[:], scratch[:, 0:128], scratch[:], start=True, stop=True)
        # sink one element so DCE keeps the warm-up matmuls (DMA'd at the end,
        # off the head-critical sync queue)
        warm_sb = cpool.tile([1, 1], F32, tag="warmsink")
        nc.vector.tensor_copy(warm_sb[:], warm_ps[0:1, 0:1])

        for b in range(B_LOC):
            qpT = projpool.tile([128, 4, Q], F16, tag="qpT")  # [p_in, p_out, q]
            kpT = projpool.tile([128, 4, K], F16, tag="kpT")
            vp = projpool.tile([128, 16, P], F16, tag="vp")  # [k_in, k_out, p]
            aT = projpool.tile([128, 4, Q], F16, tag="aT")  # [p_in, p_out, q]
            recip = cpool.tile([128, 16], F32, tag="recip")

            # ---- stage A: projections (contraction over d, 8 chunks of 128)
            # Raw q/k/v loaded in column-groups of 1024 so the first matmul
            # only waits for ~3MB and the next group prefetches during compute.
            for src_t, w_t, wsb, dest in (
                (qT_t, wq_t, wq_sb, qpT),
                (kT_t, wk_t, wk_sb, kpT),
                (vT_t, wv_t, wv_sb, vp),
            ):
                first = b == 0 and dest is qpT
                if b == 0:
                    if first:
                        # minimal first piece: the do=0 matmul waits on 128KB
                        nc.sync.dma_start(wsb[:, 0:1, :], w_t.ap()[:, 0:1, :])
                    else:
                        nc.sync.dma_start(wsb[:, 0:4, :], w_t.ap()[:, 0:4, :])
                for g in range(2):
                    rg = rawpool.tile([128, 8, 1024], F16, tag="raw")
                    src = src_t.ap()[b, :, g * 1024:(g + 1) * 1024].rearrange(
                        "(do di) s -> di do s", di=128
                    )
                    if first and g == 0:
                        nc.sync.dma_start(rg[:, 0:1, :], src[:, 0:1, :])
                        nc.sync.dma_start(wsb[:, 1:8, :], w_t.ap()[:, 1:8, :])
                        for do in range(1, 8):
                            nc.sync.dma_start(rg[:, do, :], src[:, do, :])
                    else:
                        if b == 0 and g == 0:
                            nc.sync.dma_start(wsb[:, 4:8, :], w_t.ap()[:, 4:8, :])
                        nc.sync.dma_start(rg[:, 0:4, :], src[:, 0:4, :])
                        nc.sync.dma_start(rg[:, 4:8, :], src[:, 4:8, :])
                    if dest is vp:
                        # vp[k, p]: lhsT = vT[d, k] chunk, rhs = WvT[d, p]
                        for kc8 in range(8):
                            kc = g * 8 + kc8
                            acc = ps.tile([128, 512], F32, tag="ps")
                            for do in range(8):
                                nc.tensor.matmul(
                                    acc[:],
                                    rg[:, do, kc8 * 128:(kc8 + 1) * 128],
                                    wsb[:, do, :],
                                    start=(do == 0),
                                    stop=(do == 7),
                                )
                            nc.vector.tensor_copy(vp[:, kc, :], acc[:])
                    else:
                        # qpT/kpT [p, s]: lhsT = W^T[d, p] chunk, rhs = x^T[d, s]
                        for qc2 in range(2):
                            qc = g * 2 + qc2
                            for p4 in range(4):
                                acc = ps.tile([128, 512], F32, tag="ps")
                                for do in range(8):
                                    nc.tensor.matmul(
                                        acc[:],
                                        wsb[:, do, p4 * 128:(p4 + 1) * 128],
                                        rg[:, do, qc2 * 512:(qc2 + 1) * 512],
                                        start=(do == 0),
                                        stop=(do == 7),
                                    )
                                nc.vector.tensor_copy(
                                    dest[:, p4, qc * 512:(qc + 1) * 512], acc[:]
                                )

            if b == 0:
                nc.sync.dma_start(mb_sb[:], mb_t.ap())
                nc.sync.dma_start(wo_sb[:], wo_t.ap())

            # ---- stage B: scores+softmax+PV+outproj, in two q-halves
            for h in range(2):
                q0 = h * 1024
                expT = exppool.tile([128, 16, 1024], F16, tag="expT")  # [k_in, k_out, q]

                # scores sT[k, q] + fused exp(scale*s + mask_bias[k]); DVE
                # accumulates the k_out partial sums for the softmax denom
                partial = partpool.tile([128, 1024], F32, tag="partial")
                for kc in range(16):
                    for qc in range(2):
                        s = ps.tile([128, 512], F32, tag="ps")
                        for p4 in range(4):
                            nc.tensor.matmul(
                                s[:],
                                kpT[:, p4, kc * 128:(kc + 1) * 128],
                                qpT[:, p4, q0 + qc * 512: q0 + (qc + 1) * 512],
                                start=(p4 == 0),
                                stop=(p4 == 3),
                            )
                        e_slice = expT[:, kc, qc * 512:(qc + 1) * 512]
                        nc.scalar.activation(
                            e_slice,
                            s[:],
                            Exp,
                            bias=mb_sb[:, b * 16 + kc: b * 16 + kc + 1],
                            scale=SCALE,
                        )
                        p_slice = partial[:, qc * 512:(qc + 1) * 512]
                        if kc == 0:
                            nc.vector.tensor_copy(p_slice, e_slice)
                        else:
                            nc.vector.tensor_add(p_slice, p_slice, e_slice)

                # PV: aT[p, q] = sum_k vp[k, p]^T exp[k, q]
                for p4 in range(4):
                    for qc in range(2):
                        a = pa.tile([128, 512], F32, tag="pa")
                        for kc in range(16):
                            nc.tensor.matmul(
                                a[:],
                                vp[:, kc, p4 * 128:(p4 + 1) * 128],
                                expT[:, kc, qc * 512:(qc + 1) * 512],
                                start=(kc == 0),
                                stop=(kc == 15),
                            )
                        nc.vector.tensor_copy(aT[:, p4, q0 + qc * 512: q0 + (qc + 1) * 512], a[:])

                # denom[q]: one matmul per 128-q chunk sums partial's 128
                # k_in partitions (lhsT=partial chunk, rhs=ones)
                for qm in range(8):
                    dn = pd.tile([128, 1], F32, tag="pd")
                    nc.tensor.matmul(
                        dn[:],
                        partial[:, qm * 128:(qm + 1) * 128],
                        ones_sb[:],
                        start=True,
                        stop=True,
                    )
                    nc.vector.reciprocal(recip[:, h * 8 + qm: h * 8 + qm + 1], dn[:])

                # outproj: out[q, d] = (aT[p, q]^T @ WoutT[p, d]) * recip[q]
                for qm8 in range(8):
                    qm = h * 8 + qm8
                    osb = outpool.tile([128, D], F32, tag="osb")
                    for dc in range(2):
                        o = ps.tile([128, 512], F32, tag="ps")
                        for p4 in range(4):
                            nc.tensor.matmul(
                                o[:],
                                aT[:, p4, qm * 128:(qm + 1) * 128],
                                wo_sb[:, p4, dc * 512:(dc + 1) * 512],
                                start=(p4 == 0),
                                stop=(p4 == 3),
                            )
                        nc.vector.tensor_scalar_mul(
                            osb[:, dc * 512:(dc + 1) * 512], o[:], recip[:, qm: qm + 1]
                        )
                        # per-dc DMA so the kernel-tail chain after the last
                        # matmul is one normalize + one 256KB transfer
                        nc.sync.dma_start(
                            out_t.ap()[b, qm * 128:(qm + 1) * 128, dc * 512:(dc + 1) * 512],
                            osb[:, dc * 512:(dc + 1) * 512],
                        )

        nc.sync.dma_start(scr_t.ap()[:], warm_sb[:])


def _build():
    if "nc" in _cached:
        return _cached["nc"]
    nc = bacc.Bacc(
        "TRN2",
        target_bir_lowering=False,
        debug=False,
        enable_asserts=False,
        num_devices=N_CORES,
    )
    with tile.TileContext(nc) as tc:
        _emit(nc, tc)
    nc.compile()
    _cached["nc"] = nc
    return nc


def kernel(q, k, v, mask, Wq, Wk, Wv, Wout, profile=False):
    global last_exec_time_ns
    import os

    profile = profile or bool(os.environ.get("KERNEL_PROFILE"))

    q = np.asarray(q)
    k = np.asarray(k)
    v = np.asarray(v)
    mask = np.asarray(mask)

    qT = np.ascontiguousarray(q.transpose(0, 2, 1)).astype(np.float16)
    kT = np.ascontiguousarray(k.transpose(0, 2, 1)).astype(np.float16)
    vT = np.ascontiguousarray(v.transpose(0, 2, 1)).astype(np.float16)

    def block_w(w):  # W [P_out, D_in] -> SBUF layout [128 d_in, 8 d_out, P_out]
        return np.ascontiguousarray(
            np.asarray(w).T.reshape(8, 128, w.shape[0]).transpose(1, 0, 2)
        ).astype(np.float16)

    wq = block_w(Wq)
    wk = block_w(Wk)
    wv = block_w(Wv)
    # Wout [D, P] -> WoutT [P, D] -> [128 p_in, 4 p_out, D]
    wo = np.ascontiguousarray(
        np.asarray(Wout).T.reshape(4, 128, D).transpose(1, 0, 2)
    ).astype(np.float16)
    # mask bias [B, K] -> per-batch [128, 16] (partition = k % 128, col = k // 128)
    mbias = np.where(mask, np.float32(0.0), np.float32(MASK_NEG)).astype(np.float32)
    mb = np.ascontiguousarray(mbias.reshape(B, 16, 128).transpose(0, 2, 1))  # [B,128,16]

    nc = _build()

    in_maps = []
    for c in range(N_CORES):
        sl = slice(c * B_LOC, (c + 1) * B_LOC)
        in_maps.append(
            {
                "qT": qT[sl],
                "kT": kT[sl],
                "vT": vT[sl],
                "wq": wq,
                "wk": wk,
                "wv": wv,
                "wo": wo,
                "mb": np.ascontiguousarray(
                    np.concatenate([mb[i] for i in range(sl.start, sl.stop)], axis=1)
                ),
            }
        )

    try:
        res = run_bass_kernel_spmd(nc, in_maps, list(range(N_CORES)), trace=profile)
    except Exception:
        if not profile:
            raise
        # profiling needs the NTFF hook, which not every image ships — the
        # run itself works fine without tracing
        res = run_bass_kernel_spmd(nc, in_maps, list(range(N_CORES)), trace=False)
    last_exec_time_ns = res.exec_time_ns

    out = np.concatenate([res.results[c]["out"] for c in range(N_CORES)], axis=0)
    return out.astype(np.float32)
